# revision 1
# baseline (speedup 1.0000x reference)
"""Trainium2 Bass kernel for the FFT-block (attention + conv FFN) problem.

Sharding: data-parallel over batch. B=16 items across 8 cores -> 2 items/core.

v2 changes vs baseline:
  - heads processing is software-pipelined at the ti level: ctx matmuls of
    group g interleave with scores matmuls of group g+1 so the PE never
    head-of-line blocks on the scalar-engine exp; scores for the two heads of
    a pair go to disjoint PE row-groups (64-row tiles) and run concurrently.
  - both heads' scores of a ti-slot land in one 2-bank PSUM tile and are
    consumed by a single [128,1024] exp.
  - V bias is applied by the DVE copy (tensor_tensor add) instead of a K=1
    matmul; the softmax-Z ones column comes from the bias tile.
  - the two 1/Z broadcast matmuls of a group pack into PE col-groups.
  - LN2 is pipelined per s-chunk into the last conv chunk (short tail).
  - x/Wv DMAs issue first so the PE starts sooner.
"""
import sys, types
import numpy as np

B, S, D = 16, 1024, 512
H, DK = 8, 64
CD, KS = 2048, 9
EPS = 1e-5
NCORES = 8
NIT = B // NCORES
NDC = D // 128             # 4 d-chunks
NSC = S // 128             # 8 s-chunks
NCOL = S // 512            # 2 s-cols
NCD = CD // 128            # 16 cd-chunks


def _install_ntff_hook():
    try:
        from antenv.axon_hooks import get_axon_ntff_profile_hook  # noqa
        return
    except ImportError:
        pass
    try:
        from trn_agent_boot.trn_boot import _ntff_profile_via_ctypes
        mod = types.ModuleType('antenv.axon_hooks')
        hook = _ntff_profile_via_ctypes('/opt/axon/libaxon_pjrt.so')
        mod.get_axon_ntff_profile_hook = lambda: hook
        sys.modules['antenv.axon_hooks'] = mod
    except Exception:
        pass


_BUILT = {}


def _build(affine1=False, affine2=False):
    """affine1/affine2: apply LN1/LN2 gamma,beta (skipped when g==1, b==0)."""
    global _BUILT
    key = (affine1, affine2)
    if key in _BUILT:
        return _BUILT[key]
    _install_ntff_hook()
    import concourse.bacc as bacc
    import concourse.mybir as mybir
    from concourse import tile
    from concourse.masks import make_identity
    from contextlib import ExitStack

    F32 = mybir.dt.float32
    F32R = mybir.dt.float32r
    BF16 = mybir.dt.bfloat16
    AF = mybir.ActivationFunctionType
    ALU = mybir.AluOpType
    AX = mybir.AxisListType

    nc = bacc.Bacc("TRN2", target_bir_lowering=False, debug=False,
                   num_devices=NCORES)

    # ---- DRAM I/O (per core) ----
    d_xT = nc.dram_tensor("xT", [NIT, NDC, 128, S], F32, kind="ExternalInput")
    d_xp = nc.dram_tensor("xp", [NIT, NSC, 128, D], F32, kind="ExternalInput")
    d_wqk = nc.dram_tensor("wqk", [2, 4, 128, 512], F32, kind="ExternalInput")
    d_bqk = nc.dram_tensor("bqk", [128, 8], F32, kind="ExternalInput")
    d_wv = nc.dram_tensor("wv", [NDC, 128, 520], F32, kind="ExternalInput")
    d_bvrow = nc.dram_tensor("bvrow", [128, 520], F32, kind="ExternalInput")
    d_wo = nc.dram_tensor("wo", [4, 128, 512], F32, kind="ExternalInput")
    d_w1 = nc.dram_tensor("w1", [NCD, NDC, 128, KS * 128], BF16,
                          kind="ExternalInput")
    d_w2 = nc.dram_tensor("w2", [NCD, 128, KS * 512], BF16,
                          kind="ExternalInput")
    d_bc1s = nc.dram_tensor("bc1s", [128, NCD], F32, kind="ExternalInput")
    d_gb = nc.dram_tensor("gb", [5, 128, 512], F32, kind="ExternalInput")
    d_cones = nc.dram_tensor("cones", [128, 128], F32, kind="ExternalInput")
    d_czero = nc.dram_tensor("czero", [128, 8], BF16, kind="ExternalInput")
    d_y = nc.dram_tensor("y", [NIT, NSC, 128, D], F32, kind="ExternalOutput")

    G1, B1, G2, B2, BC2 = range(5)

    with tile.TileContext(nc) as tc:
        est = ExitStack()
        with est:
            cp = est.enter_context(tc.tile_pool(name="const", bufs=1))
            pl = est.enter_context(tc.tile_pool(name="work", bufs=1))
            ps = est.enter_context(tc.tile_pool(name="psum", bufs=1, space="PSUM"))
            dp = est.enter_context(tc.tile_pool(name="dramp", bufs=1, space="DRAM"))

            h_dram = [[dp.tile([128, D], F32, tag=f"hd{it}_{sc}",
                               name=f"hd{it}_{sc}")
                       for sc in range(NSC)] for it in range(NIT)]

            state = [dict() for _ in range(NIT)]

            # ---- input x first (feeds the first matmuls) ----
            def emit_x(it):
                st = state[it]
                xt = []
                for dc in range(NDC):
                    t = pl.tile([128, S], F32R, tag=f"xt{dc}", name=f"xt{dc}")
                    nc.sync.dma_start(t[:], d_xT[it, dc].bitcast(F32R))
                    xt.append(t)
                st["xt"] = xt
                st["qkt"] = {}

            emit_x(0)

            # ---- constants (V-projection ones first) ----
            t_wv = []
            for dc in range(NDC):
                t = cp.tile([128, 520], F32R, tag=f"wv{dc}", name=f"wv{dc}")
                nc.sync.dma_start(t[:], d_wv[dc].bitcast(F32R))
                t_wv.append(t)
            t_bvfull = cp.tile([128, 520], F32, tag="bvfull")
            nc.sync.dma_start(t_bvfull[:], d_bvrow[:])
            t_bqk = cp.tile([128, 8], F32, tag="bqk")
            nc.sync.dma_start(t_bqk[:], d_bqk[:])
            t_cones = cp.tile([128, 128], F32R, tag="cones")
            nc.sync.dma_start(t_cones[:], d_cones[:].bitcast(F32R))
            t_gb = []
            for i in range(5):
                t = cp.tile([128, 512], F32, tag=f"gb{i}", name=f"gb{i}")
                nc.sync.dma_start(t[:], d_gb[i])
                t_gb.append(t)
            t_bc1s = cp.tile([128, NCD], F32, tag="bc1s")
            nc.sync.dma_start(t_bc1s[:], d_bc1s[:])
            t_ident = cp.tile([128, 128], F32, tag="ident")
            make_identity(nc, t_ident[:])
            t_czero = cp.tile([128, 8], BF16, tag="czero")
            nc.sync.dma_start(t_czero[:], d_czero[:])
            t_eps = cp.tile([128, 1], F32, tag="eps")
            nc.vector.memset(t_eps[:], EPS)
            t_wo = []
            for c in range(4):
                t = cp.tile([128, 512], F32R, tag=f"wo{c}", name=f"wo{c}")
                nc.sync.dma_start(t[:], d_wo[c].bitcast(F32R))
                t_wo.append(t)

            # persistent hT tiles (bf16, padded s)
            hT = [[pl.tile([128, S + 8], BF16, tag=f"ht{it}_{dc}",
                           name=f"ht{it}_{dc}")
                   for dc in range(NDC)] for it in range(NIT)]

            # ================= emit helpers =================
            def emit_v(it):
                """V projection for one item; bias + Z-ones via DVE add."""
                st = state[it]
                xt = st["xt"]
                vst = []
                for tc_i in range(NSC):
                    vt = pl.tile([128, 520], BF16, tag=f"vst{tc_i}",
                                 name=f"vst{tc_i}")
                    for half in range(2):
                        colo = half * 260
                        pv = ps.tile([128, 260], F32, tag="pp", bufs=2)
                        for dc in range(NDC):
                            nc.tensor.matmul(
                                pv[:], xt[dc][:, tc_i * 128:(tc_i + 1) * 128],
                                t_wv[dc][:, colo:colo + 260],
                                start=(dc == 0), stop=(dc == NDC - 1))
                        nc.vector.tensor_tensor(
                            vt[:, colo:colo + 260], pv[:],
                            t_bvfull[:, colo:colo + 260], ALU.add)
                    vst.append(vt)
                st["vst"] = vst

            def emit_qk(it, pair):
                st = state[it]
                xt = st["xt"]
                for proj in range(2):
                    wt = pl.tile([128, 512], F32R, tag=f"wqk{proj}",
                                 bufs=2, name="wt")
                    nc.sync.dma_start(wt[:], d_wqk[proj, pair].bitcast(F32R))
                    qt = pl.tile([128, S], BF16, tag=f"qk{proj}{pair}",
                                 name="qt")
                    for scol in range(NCOL):
                        pq = ps.tile([128, 512], F32, tag="pp", bufs=2)
                        for dc in range(NDC):
                            nc.tensor.matmul(
                                pq[:], wt[:, dc * 128:(dc + 1) * 128],
                                xt[dc][:, scol * 512:(scol + 1) * 512],
                                start=(dc == 0), stop=(dc == NDC - 1))
                        nc.vector.tensor_scalar_add(
                            qt[:, scol * 512:(scol + 1) * 512], pq[:],
                            t_bqk[:, proj * 4 + pair:proj * 4 + pair + 1])
                    st["qkt"][(proj, pair)] = qt

            def heads_gen(it):
                """Pipelined heads processing; yields once per emission
                quantum so the driver can interleave other work."""
                st = state[it]
                st["ctxT"] = [pl.tile([128, S], F32R, tag=f"ct{c}",
                                      name=f"ct{c}") for c in range(4)]
                ctxT = st["ctxT"]
                vst = st["vst"]
                groups = [(p, s) for p in range(4) for s in range(2)]
                pex = {}          # ti -> [128,1024] bf16 tile (2 heads)
                pc = {}           # group -> [pc0, pc1]

                def scores_slot(g, ti):
                    pair, scol = g
                    so = scol * 512
                    qT = st["qkt"][(0, pair)]
                    kT = st["qkt"][(1, pair)]
                    sc2 = ps.tile([128, 1024], F32, tag="sc2", bufs=2,
                                  name="sc2")
                    for sub in range(2):
                        hr = slice(sub * 64, sub * 64 + 64)
                        nc.tensor.matmul(
                            sc2[:, sub * 512:(sub + 1) * 512],
                            kT[hr, ti * 128:(ti + 1) * 128],
                            qT[hr, so:so + 512], start=True, stop=True)
                    pe = pl.tile([128, 1024], BF16, tag=f"pex{ti}", bufs=1,
                                 name="pe")
                    nc.scalar.activation(pe[:], sc2[:], AF.Exp, scale=0.125)
                    pex[ti] = pe

                def ctx_slot(g, ti):
                    pair, scol = g
                    if ti == 0:
                        pc[g] = [ps.tile([65, 512], F32, tag="pc", bufs=2,
                                         name=f"pcx{sub}")
                                 for sub in range(2)]
                    for sub in range(2):
                        h = 2 * pair + sub
                        nc.tensor.matmul(
                            pc[g][sub][:], vst[ti][:, h * 65:h * 65 + 65],
                            pex[ti][:, sub * 512:(sub + 1) * 512],
                            start=(ti == 0), stop=(ti == NSC - 1))

                def norm(g):
                    pair, scol = g
                    so = scol * 512
                    for sub in range(2):
                        hr = slice(sub * 64, sub * 64 + 64)
                        zr = pl.tile([1, 512], F32R, tag="bcs", bufs=4,
                                     name="zr")
                        nc.vector.tensor_copy(zr[0:1, :], pc[g][sub][64:65, :])
                        pb = ps.tile([64, 512], F32, tag="pp", bufs=2,
                                     name="pb")
                        nc.tensor.matmul(pb[:], t_cones[0:1, 0:64],
                                         zr[0:1, :], start=True, stop=True)
                        bcs = pl.tile([64, 512], F32, tag="bcs", bufs=4,
                                      name="bcs")
                        nc.vector.reciprocal_approx_fast(out=bcs[:], in_=pb[:])
                        nc.vector.tensor_tensor(
                            ctxT[pair][hr, so:so + 512], pc[g][sub][0:64, :],
                            bcs[:], ALU.mult)
                    del pc[g]

                for ti in range(NSC):
                    scores_slot(groups[0], ti)
                    yield ("pro", ti)
                for gi, g in enumerate(groups):
                    nxt = groups[gi + 1] if gi + 1 < len(groups) else None
                    for ti in range(NSC):
                        ctx_slot(g, ti)
                        if nxt is not None:
                            scores_slot(nxt, ti)
                        yield ("slot", gi, ti)
                    norm(g)
                    yield ("norm", gi)

            def emit_tail(it):
                """Wo + residual + LN1 + transpose into hT (+ h spill),
                pipelined per s-chunk; one-pass bn_stats for mean/var."""
                st = state[it]
                ctxT = st["ctxT"]
                for sc in range(NSC):
                    xpt = pl.tile([128, 512], F32, tag="xpt", bufs=2)
                    nc.sync.dma_start(xpt[:], d_xp[it, sc])
                    pw = ps.tile([128, 512], F32, tag="pc", bufs=2)
                    for c in range(4):
                        nc.tensor.matmul(
                            pw[:], ctxT[c][:, sc * 128:(sc + 1) * 128],
                            t_wo[c][:], start=(c == 0), stop=(c == 3))
                    r = pl.tile([128, 512], F32, tag="res", bufs=3, name="r")
                    nc.vector.tensor_tensor(r[:], pw[:], xpt[:], ALU.add)
                    st6 = pl.tile([128, 6], F32, tag="st6", bufs=2)
                    mv = pl.tile([128, 2], F32, tag="mv", bufs=2)
                    nc.vector.bn_stats(st6[:], r[:])
                    nc.vector.bn_aggr(mv[:], st6[:])
                    inv = pl.tile([128, 1], F32, tag="st4", bufs=2)
                    nc.scalar.activation(inv[:], mv[:, 1:2], AF.Sqrt,
                                         bias=t_eps[:])
                    nc.vector.reciprocal(inv[:], inv[:])
                    ht_ = pl.tile([128, 512], F32, tag="hst", bufs=2, name="h_")
                    nc.vector.tensor_scalar(
                        ht_[:], r[:], mv[:, 0:1], inv[:],
                        ALU.subtract, ALU.mult)
                    if affine1:
                        nc.vector.tensor_tensor(ht_[:], ht_[:], t_gb[G1][:],
                                                ALU.mult)
                        nc.vector.tensor_tensor(ht_[:], ht_[:], t_gb[B1][:],
                                                ALU.add)
                    nc.sync.dma_start(h_dram[it][sc][:], ht_[:])
                    for dc in range(NDC):
                        pt = ps.tile([128, 128], F32, tag="sc2", bufs=2,
                                     name="pt")
                        nc.tensor.transpose(pt[:], ht_[:, dc * 128:(dc + 1) * 128],
                                            t_ident[:])
                        nc.scalar.copy(
                            hT[it][dc][:, 4 + sc * 128: 4 + (sc + 1) * 128],
                            pt[:])
                for dc in range(NDC):
                    nc.sync.dma_start(hT[it][dc][:, 0:4], d_czero[:, 0:4])
                    nc.sync.dma_start(hT[it][dc][:, S + 4:S + 8],
                                      d_czero[:, 4:8])

            o2 = [[None] * NSC for _ in range(NIT)]

            def emit_ln2_sc(it, sc, pc2):
                """Final-chunk conv2 psum -> +o2, relu, +h, LN2, out.
                bc2 is folded into o2's chunk-0 init."""
                t1 = pl.tile([128, 512], F32, tag="hst", bufs=2)
                nc.vector.tensor_tensor(t1[:], pc2[:], o2[it][sc][:], ALU.add)
                nc.scalar.activation(t1[:], t1[:], AF.Relu)
                hrl = pl.tile([128, 512], F32, tag="xpt", bufs=2)
                nc.sync.dma_start(hrl[:], h_dram[it][sc][:])
                r = pl.tile([128, 512], F32, tag="res", bufs=3, name="r2")
                nc.vector.tensor_tensor(r[:], t1[:], hrl[:], ALU.add)
                st6 = pl.tile([128, 6], F32, tag="st6", bufs=2)
                mv = pl.tile([128, 2], F32, tag="mv", bufs=2)
                nc.vector.bn_stats(st6[:], r[:])
                nc.vector.bn_aggr(mv[:], st6[:])
                inv = pl.tile([128, 1], F32, tag="st4", bufs=2)
                nc.scalar.activation(inv[:], mv[:, 1:2], AF.Sqrt,
                                     bias=t_eps[:])
                nc.vector.reciprocal(inv[:], inv[:])
                yt = pl.tile([128, 512], F32, tag="hst", bufs=2)
                nc.vector.tensor_scalar(yt[:], r[:], mv[:, 0:1], inv[:],
                                        ALU.subtract, ALU.mult)
                if affine2:
                    nc.vector.tensor_tensor(yt[:], yt[:], t_gb[G2][:],
                                            ALU.mult)
                    nc.vector.tensor_tensor(yt[:], yt[:], t_gb[B2][:], ALU.add)
                nc.sync.dma_start(d_y[it, sc], yt[:])

            def emit_conv_chunk(it, cdc, pull=None):
                last = (cdc == NCD - 1)
                w2t = pl.tile([128, KS * 512], BF16, tag="w2t", bufs=2,
                              name="w2t")
                nc.sync.dma_start(w2t[:], d_w2[cdc])
                w1t = []
                for dc in range(NDC):
                    t = pl.tile([128, KS * 128], BF16, tag=f"w1t{dc}", bufs=2,
                                name="w1t")
                    nc.sync.dma_start(t[:], d_w1[cdc, dc])
                    w1t.append(t)
                c1t = pl.tile([128, S + 8], BF16, tag="c1t", bufs=2, name="c1t")
                nc.sync.dma_start(c1t[:, 0:4], d_czero[:, 0:4])
                nc.sync.dma_start(c1t[:, S + 4:S + 8], d_czero[:, 4:8])
                for scol in range(NCOL):
                    pc1 = ps.tile([128, 512], F32, tag="pp", bufs=2)
                    idx = 0
                    for k in range(KS):
                        for dc in range(NDC):
                            nc.tensor.matmul(
                                pc1[:], w1t[dc][:, k * 128:(k + 1) * 128],
                                hT[it][dc][:, scol * 512 + k:
                                           scol * 512 + k + 512],
                                start=(idx == 0), stop=(idx == 35))
                            idx += 1
                    nc.scalar.activation(
                        c1t[:, 4 + scol * 512: 4 + (scol + 1) * 512],
                        pc1[:], AF.Relu, bias=t_bc1s[:, cdc:cdc + 1])
                    if pull is not None:
                        pull()
                for sc in range(NSC):
                    pc2 = ps.tile([128, 512], F32, tag="pp", bufs=2)
                    for k in range(KS):
                        nc.tensor.matmul(
                            pc2[:], c1t[:, sc * 128 + k: sc * 128 + k + 128],
                            w2t[:, k * 512:(k + 1) * 512],
                            start=(k == 0), stop=(k == KS - 1))
                    if cdc == 0:
                        t = pl.tile([128, 512], F32, tag=f"o2_{sc}",
                                    name=f"o2_{sc}")
                        o2[it][sc] = t
                        nc.vector.tensor_tensor(t[:], pc2[:], t_gb[BC2][:],
                                                ALU.add)
                    elif not last:
                        nc.vector.tensor_tensor(o2[it][sc][:], pc2[:],
                                                o2[it][sc][:], ALU.add)
                    else:
                        emit_ln2_sc(it, sc, pc2)
                    if pull is not None:
                        pull()

            # ================= emission order =================
            emit_v(0)
            for pair in range(4):
                emit_qk(0, pair)
            emit_x(1)
            for tok in heads_gen(0):
                if tok[0] == "norm" and tok[1] % 2 == 1:
                    emit_qk(1, tok[1] // 2)
            emit_v(1)
            emit_tail(0)
            hg1 = heads_gen(1)

            def pull():
                next(hg1, None)

            for cdc in range(NCD):
                emit_conv_chunk(0, cdc, pull=pull if cdc < 8 else None)
                if cdc == 7:
                    for _ in hg1:        # drain any remaining heads slots
                        pass
                    emit_tail(1)
            for cdc in range(NCD):
                emit_conv_chunk(1, cdc)

    nc.compile()
    _BUILT[key] = nc
    return nc


def _prep_host(inputs):
    import ml_dtypes
    bf16 = ml_dtypes.bfloat16
    x = np.asarray(inputs["x"], np.float32)
    Wq = np.asarray(inputs["Wq"], np.float32)
    bq = np.asarray(inputs["bq"], np.float32)
    Wk = np.asarray(inputs["Wk"], np.float32)
    bk = np.asarray(inputs["bk"], np.float32)
    Wv = np.asarray(inputs["Wv"], np.float32)
    bv = np.asarray(inputs["bv"], np.float32)
    Wo = np.asarray(inputs["Wo"], np.float32)
    bo = np.asarray(inputs["bo"], np.float32)
    g1 = np.asarray(inputs["g1"], np.float32)
    b1 = np.asarray(inputs["b1"], np.float32)
    g2 = np.asarray(inputs["g2"], np.float32)
    b2 = np.asarray(inputs["b2"], np.float32)
    Wc1 = np.asarray(inputs["Wc1"], np.float32)
    bc1 = np.asarray(inputs["bc1"], np.float32)
    Wc2 = np.asarray(inputs["Wc2"], np.float32)
    bc2 = np.asarray(inputs["bc2"], np.float32)

    xT = np.ascontiguousarray(x.transpose(0, 2, 1).reshape(B, NDC, 128, S))
    xp = np.ascontiguousarray((x + bo[None, None, :]).reshape(B, NSC, 128, D))

    wqk = np.zeros((2, 4, 128, 512), np.float32)
    for proj, W in ((0, Wq), (1, Wk)):
        for pair in range(4):
            blk = np.concatenate([W[2 * pair], W[2 * pair + 1]], axis=1)
            wqk[proj, pair] = blk.reshape(NDC, 128, 128).transpose(1, 0, 2) \
                                 .reshape(128, 512)
    bqk = np.zeros((128, 8), np.float32)
    for proj, b in ((0, bq), (1, bk)):
        for pair in range(4):
            bqk[:, proj * 4 + pair] = np.concatenate(
                [b[2 * pair], b[2 * pair + 1]])

    wv = np.zeros((NDC, 128, 520), np.float32)
    bvrow = np.zeros((128, 520), np.float32)
    for h in range(H):
        wv[:, :, h * 65:h * 65 + 64] = Wv[h].reshape(NDC, 128, 64)
        bvrow[:, h * 65:h * 65 + 64] = bv[h][None, :]
        bvrow[:, h * 65 + 64] = 1.0

    wo = np.ascontiguousarray(Wo.reshape(4, 128, 512))

    w1 = np.ascontiguousarray(
        Wc1.reshape(NCD, 128, NDC, 128, KS).transpose(0, 2, 3, 4, 1)
           .reshape(NCD, NDC, 128, KS * 128)).astype(bf16)
    w2 = np.ascontiguousarray(
        Wc2.reshape(D, NCD, 128, KS).transpose(1, 2, 3, 0)
           .reshape(NCD, 128, KS * 512)).astype(bf16)
    bc1s = np.ascontiguousarray(bc1.reshape(NCD, 128).T)

    gb = np.stack([np.tile(v[None, :], (128, 1))
                   for v in (g1, b1, g2, b2, bc2)]).astype(np.float32)
    cones = np.ones((128, 128), np.float32)
    czero = np.zeros((128, 8), bf16)

    shared = dict(wqk=wqk, bqk=bqk, wv=wv, bvrow=bvrow, wo=wo,
                  w1=w1, w2=w2, bc1s=bc1s, gb=gb, cones=cones, czero=czero)
    in_maps = []
    for c in range(NCORES):
        m = dict(shared)
        m["xT"] = np.ascontiguousarray(xT[c * NIT:(c + 1) * NIT])
        m["xp"] = np.ascontiguousarray(xp[c * NIT:(c + 1) * NIT])
        in_maps.append(m)
    return in_maps


def run(inputs, trace=False, **trace_kwargs):
    affine1 = not (np.all(np.asarray(inputs["g1"]) == 1.0)
                   and np.all(np.asarray(inputs["b1"]) == 0.0))
    affine2 = not (np.all(np.asarray(inputs["g2"]) == 1.0)
                   and np.all(np.asarray(inputs["b2"]) == 0.0))
    nc = _build(affine1, affine2)
    from concourse.bass_utils import run_bass_kernel_spmd
    in_maps = _prep_host(inputs)
    res = run_bass_kernel_spmd(nc, in_maps, core_ids=list(range(NCORES)),
                               trace=trace, **trace_kwargs)
    y = np.concatenate([res.results[c]["y"].reshape(NIT, S, D)
                        for c in range(NCORES)], axis=0)
    return y, res


def kernel(**inputs):
    y, _ = run(inputs, trace=False)
    return y

